# revision 1
# baseline (speedup 1.0000x reference)
"""GNN message-passing (DGL-style ConvLayer) Trainium2 Bass kernel.

Strategy (8 NeuronCores, full inputs in / full output out):
  - Destination nodes are sharded: core c owns dst rows [c*6250, (c+1)*6250).
  - Host groups edges by (core, dst_block_of_128) and sorts by src within a
    block, laying them into a fixed slot grid [128 partitions x K_MAX tiles]
    per block (padding slots use src=0 / dst_local=-1).
  - Per block, one bulk indirect DMA gathers h_neigh[src] rows (512B each)
    near HBM line rate; edge feats are DMA'd into the same interleaved
    [128, K_MAX, 161] tile (cols 0:128 gather, 128:160 edge feats, 160 ones).
  - Segment-sum over dst is a PE matmul with a one-hot selection matrix S
    (S[e,d] = dst_local[e]==d) built on DVE via is_equal against an iota
    tile; the ones column yields per-dst degree in the same accumulation.
  - Epilogue per block: scale by 1/max(deg,1), transpose, project with
    replicated W_self/W_neigh, relu, and row L2-normalize; DMA rows out.

No collectives needed: each core owns its dst rows end to end.
"""
import math
import os
import numpy as np

import concourse.bass as bass
import concourse.bacc as bacc
import concourse.mybir as mybir
import concourse.tile as tile

N_SRC = 50000
N_DST = 50000
D_NEIGH = 128
D_EDGE = 32
D_OUT = 256
N_CORES = 8
DST_PER_CORE = N_DST // N_CORES  # 6250
P = 128
N_BLOCKS = math.ceil(DST_PER_CORE / P)  # 49 blocks of 128 dst (6272 padded)
DST_PAD = N_BLOCKS * P  # 6272
F_TOT = D_NEIGH + D_EDGE + 1  # 161: gathered | edge feats | ones


def _maybe_install_trace_hooks():
    """Only used when BASS_TRACE is set (dev/profiling); recreates the NTFF
    hook missing from this image and no-ops the artifact upload."""
    if not os.environ.get("BASS_TRACE"):
        return
    import contextlib
    import ctypes
    import sys
    import types

    if "antenv.axon_hooks" in sys.modules:
        return
    try:
        lib = ctypes.CDLL("/opt/axon/libaxon_pjrt.so")
        lib.axon_start_nrt_profile.argtypes = [
            ctypes.POINTER(ctypes.c_int64),
            ctypes.c_size_t,
        ]
        lib.axon_start_nrt_profile.restype = ctypes.c_int64
        lib.axon_stop_nrt_profile.argtypes = [ctypes.c_char_p]
        lib.axon_stop_nrt_profile.restype = ctypes.c_int64
    except OSError:
        return

    @contextlib.contextmanager
    def _hook(output_dir, device_ids=None):
        import jax

        jax.devices()
        if device_ids:
            ids = (ctypes.c_int64 * len(device_ids))(*device_ids)
            rc = lib.axon_start_nrt_profile(ids, len(device_ids))
        else:
            rc = lib.axon_start_nrt_profile(None, 0)
        if rc != 0:
            raise RuntimeError(f"axon_start_nrt_profile rc={rc}")
        try:
            yield
        finally:
            n = lib.axon_stop_nrt_profile(str(output_dir).encode())
            print(f"ntff profile: {n} file(s) -> {output_dir}", file=sys.stderr)

    mod = types.ModuleType("antenv.axon_hooks")
    mod.get_axon_ntff_profile_hook = lambda: _hook
    mod.set_axon_ntff_profile_hook = lambda h: None
    sys.modules["antenv.axon_hooks"] = mod

    import concourse.bass_utils as bu

    bu.upload_artifacts = lambda tmpdir: tmpdir


def build_program(k_max: int):
    """Build the SPMD Bass program (identical across cores)."""
    nc = bacc.Bacc("TRN2", target_bir_lowering=False, debug=False,
                   num_devices=N_CORES)
    f32 = mybir.dt.float32
    i32 = mybir.dt.int32

    h_neigh = nc.dram_tensor("h_neigh", [N_SRC, D_NEIGH], f32, kind="ExternalInput")
    srcidx = nc.dram_tensor("srcidx", [P, N_BLOCKS * k_max], i32, kind="ExternalInput")
    dloc = nc.dram_tensor("dloc", [P, N_BLOCKS * k_max], f32, kind="ExternalInput")
    ef = nc.dram_tensor("ef", [N_BLOCKS, P, k_max * D_EDGE], f32, kind="ExternalInput")
    h_selfT = nc.dram_tensor("h_selfT", [P, DST_PAD], f32, kind="ExternalInput")
    wsT = nc.dram_tensor("wsT", [128, D_OUT], f32, kind="ExternalInput")
    wnT1 = nc.dram_tensor("wnT1", [128, D_OUT], f32, kind="ExternalInput")
    wnT2 = nc.dram_tensor("wnT2", [D_EDGE, D_OUT], f32, kind="ExternalInput")
    iota = nc.dram_tensor("iota", [P, P], f32, kind="ExternalInput")
    ident = nc.dram_tensor("ident", [P, P], f32, kind="ExternalInput")
    out = nc.dram_tensor("out", [DST_PAD, D_OUT], f32, kind="ExternalOutput")

    K = k_max
    with tile.TileContext(nc) as tc:
        with (
            tc.tile_pool(name="const", bufs=1) as cp,
            tc.tile_pool(name="t3p", bufs=3) as t3p,
            tc.tile_pool(name="sp", bufs=2) as sp,
            tc.tile_pool(name="work", bufs=2) as wp,
            tc.tile_pool(name="small", bufs=3) as smp,
            tc.tile_pool(name="pagg", bufs=2, space="PSUM") as pagg,
            tc.tile_pool(name="ptr", bufs=2, space="PSUM") as ptr,
            tc.tile_pool(name="pz", bufs=2, space="PSUM") as pz,
        ):
            # resident constants
            hsT_sb = cp.tile([P, DST_PAD], f32)
            nc.sync.dma_start(out=hsT_sb[:], in_=h_selfT[:])
            src_sb = cp.tile([P, N_BLOCKS * K], i32)
            nc.sync.dma_start(out=src_sb[:], in_=srcidx[:])
            dl_sb = cp.tile([P, N_BLOCKS * K], f32)
            nc.sync.dma_start(out=dl_sb[:], in_=dloc[:])
            wsT_sb = cp.tile([128, D_OUT], f32)
            nc.sync.dma_start(out=wsT_sb[:], in_=wsT[:])
            wnT1_sb = cp.tile([128, D_OUT], f32)
            nc.sync.dma_start(out=wnT1_sb[:], in_=wnT1[:])
            wnT2_sb = cp.tile([D_EDGE, D_OUT], f32)
            nc.sync.dma_start(out=wnT2_sb[:], in_=wnT2[:])
            iota_sb = cp.tile([P, P], f32)
            nc.sync.dma_start(out=iota_sb[:], in_=iota[:])
            ident_sb = cp.tile([P, P], f32)
            nc.sync.dma_start(out=ident_sb[:], in_=ident[:])

            for b in range(N_BLOCKS):
                # contiguous gather tile [128, K*128] + edge-feats/ones tile
                g = t3p.tile([P, K * D_NEIGH], f32, tag="g")
                # one indirect DMA per edge tile: HW honors exactly one
                # index per partition (multi-index offset APs silently
                # stream contiguous rows instead - verified on HW)
                for j in range(K):
                    nc.gpsimd.indirect_dma_start(
                        out=g[:, j * D_NEIGH : (j + 1) * D_NEIGH],
                        out_offset=None,
                        in_=h_neigh[:],
                        in_offset=bass.IndirectOffsetOnAxis(
                            ap=src_sb[:, b * K + j : b * K + j + 1], axis=0
                        ),
                    )
                efo = t3p.tile([P, K, D_EDGE + 1], f32, tag="efo")
                nc.sync.dma_start(out=efo[:, :, 0:D_EDGE], in_=ef[b])
                nc.vector.memset(efo[:, :, D_EDGE : D_EDGE + 1], 1.0)

                # one-hot selection matrices for all K tiles in one DVE op
                s_all = sp.tile([P, K, P], f32, tag="s")
                dl_b = dl_sb[:, b * K : (b + 1) * K].to_broadcast([P, K, P])
                iota_b = bass.AP(
                    iota_sb[:].tensor,
                    iota_sb[:].offset,
                    [list(iota_sb[:].ap[0]), [0, K], [1, P]],
                )
                nc.vector.tensor_tensor(
                    out=s_all[:], in0=dl_b, in1=iota_b,
                    op=mybir.AluOpType.is_equal,
                )

                # segment-sum via PE: psum_agg[d, f] += sum_e S[e,d] * F[e,f]
                # two matmuls per edge tile (shared stationary S_j):
                # cols 0:128 from the gather, cols 128:161 from ef|ones
                psum_agg = pagg.tile([P, F_TOT], f32, tag="agg")
                for j in range(K):
                    nc.tensor.matmul(
                        psum_agg[:, 0:D_NEIGH],
                        lhsT=s_all[:, j, :],
                        rhs=g[:, j * D_NEIGH : (j + 1) * D_NEIGH],
                        start=(j == 0),
                        stop=(j == K - 1),
                    )
                for j in range(K):
                    nc.tensor.matmul(
                        psum_agg[:, D_NEIGH:F_TOT],
                        lhsT=s_all[:, j, :],
                        rhs=efo[:, j, :],
                        start=(j == 0),
                        stop=(j == K - 1),
                    )

                # degree -> 1/max(deg,1); scale aggregated sums
                deg = smp.tile([P, 1], f32, tag="deg")
                nc.vector.tensor_copy(out=deg[:], in_=psum_agg[:, D_NEIGH + D_EDGE :])
                nc.vector.tensor_scalar_max(out=deg[:], in0=deg[:], scalar1=1.0)
                rdeg = smp.tile([P, 1], f32, tag="rdeg")
                nc.vector.reciprocal(out=rdeg[:], in_=deg[:])
                hn = wp.tile([P, D_NEIGH + D_EDGE], f32, tag="hn")
                nc.vector.tensor_scalar_mul(
                    out=hn[:], in0=psum_agg[:, 0 : D_NEIGH + D_EDGE], scalar1=rdeg[:]
                )

                # transpose hn for use as matmul weights
                pt1 = ptr.tile([P, P], f32, tag="pt1")
                nc.tensor.transpose(out=pt1[:], in_=hn[:, 0:128], identity=ident_sb[:])
                pt2 = ptr.tile([D_EDGE, P], f32, tag="pt2")
                nc.tensor.transpose(
                    out=pt2[:], in_=hn[:, 128 : 128 + D_EDGE], identity=ident_sb[:]
                )
                hnT1 = wp.tile([P, P], f32, tag="hnT1")
                nc.vector.tensor_copy(out=hnT1[:], in_=pt1[:])
                hnT2 = wp.tile([D_EDGE, P], f32, tag="hnT2")
                nc.vector.tensor_copy(out=hnT2[:], in_=pt2[:])

                # z = relu(h_self @ Ws.T + hn @ Wn.T)
                psum_z = pz.tile([P, D_OUT], f32, tag="z")
                nc.tensor.matmul(psum_z[:], lhsT=hnT1[:], rhs=wnT1_sb[:], start=True, stop=False)
                nc.tensor.matmul(psum_z[:], lhsT=hnT2[:], rhs=wnT2_sb[:], start=False, stop=False)
                nc.tensor.matmul(
                    psum_z[:],
                    lhsT=hsT_sb[:, b * P : (b + 1) * P],
                    rhs=wsT_sb[:],
                    start=False,
                    stop=True,
                )
                z = wp.tile([P, D_OUT], f32, tag="z_sb")
                nc.vector.tensor_scalar_max(out=z[:], in0=psum_z[:], scalar1=0.0)

                # row L2 norm (guard zero rows), scale, store
                sq = wp.tile([P, D_OUT], f32, tag="sq")
                ss = smp.tile([P, 1], f32, tag="ss")
                nc.scalar.activation(
                    out=sq[:], in_=z[:],
                    func=mybir.ActivationFunctionType.Square,
                    accum_out=ss[:],
                )
                nrm = smp.tile([P, 1], f32, tag="nrm")
                nc.scalar.sqrt(out=nrm[:], in_=ss[:])
                eq = smp.tile([P, 1], f32, tag="eq")
                nc.vector.tensor_scalar(
                    out=eq[:], in0=nrm[:], scalar1=0.0, scalar2=None,
                    op0=mybir.AluOpType.is_equal,
                )
                nc.vector.tensor_tensor(
                    out=nrm[:], in0=nrm[:], in1=eq[:], op=mybir.AluOpType.add
                )
                rnrm = smp.tile([P, 1], f32, tag="rnrm")
                nc.vector.reciprocal(out=rnrm[:], in_=nrm[:])
                o = wp.tile([P, D_OUT], f32, tag="o")
                nc.vector.tensor_scalar_mul(out=o[:], in0=z[:], scalar1=rnrm[:])
                nc.sync.dma_start(out=out[b * P : (b + 1) * P, :], in_=o[:])

    nc.compile()
    return nc


def preprocess(h_neigh, h_self, edge_feats, src, dst):
    """Lay edges into the per-core slot grid. All vectorized numpy."""
    e = src.shape[0]
    dst64 = dst.astype(np.int64)
    core = dst64 // DST_PER_CORE
    local = dst64 - core * DST_PER_CORE
    blk = local // P
    gblk = core * N_BLOCKS + blk  # 0 .. 8*49-1
    n_gblk = N_CORES * N_BLOCKS

    order = np.argsort(gblk * np.int64(N_SRC) + src, kind="stable")
    g_s = gblk[order]
    src_s = src[order]
    dl_s = (local - blk * P)[order].astype(np.float32)

    counts = np.bincount(gblk, minlength=n_gblk)
    k_max = int(math.ceil(counts.max() / P))
    starts = np.zeros(n_gblk, dtype=np.int64)
    starts[1:] = np.cumsum(counts)[:-1]
    rank = np.arange(e, dtype=np.int64) - starts[g_s]
    p = rank // k_max
    j = rank % k_max

    c_s = g_s // N_BLOCKS
    b_s = g_s % N_BLOCKS

    srcidx = np.zeros((N_CORES, P, N_BLOCKS * k_max), dtype=np.int32)
    dlarr = np.full((N_CORES, P, N_BLOCKS * k_max), -1.0, dtype=np.float32)
    efarr = np.zeros((N_CORES, N_BLOCKS, P, k_max, D_EDGE), dtype=np.float32)

    col = b_s * k_max + j
    srcidx[c_s, p, col] = src_s
    dlarr[c_s, p, col] = dl_s
    efarr[c_s, b_s, p, j, :] = edge_feats[order]
    efarr = efarr.reshape(N_CORES, N_BLOCKS, P, k_max * D_EDGE)

    hsT = np.zeros((N_CORES, P, DST_PAD), dtype=np.float32)
    hs = h_self.reshape(N_CORES, DST_PER_CORE, D_NEIGH)
    for c in range(N_CORES):
        hsT[c, :, :DST_PER_CORE] = hs[c].T

    return k_max, srcidx, dlarr, efarr, hsT


_PROGRAM_CACHE = {}
LAST_EXEC_NS = None


def kernel(h_neigh, h_self, edge_feats, src, dst, W_self, W_neigh):
    global LAST_EXEC_NS
    _maybe_install_trace_hooks()
    from concourse.bass_utils import run_bass_kernel_spmd

    h_neigh = np.ascontiguousarray(h_neigh, dtype=np.float32)
    h_self = np.ascontiguousarray(h_self, dtype=np.float32)
    edge_feats = np.ascontiguousarray(edge_feats, dtype=np.float32)
    src = np.ascontiguousarray(src, dtype=np.int32)
    dst = np.ascontiguousarray(dst, dtype=np.int32)
    W_self = np.ascontiguousarray(W_self, dtype=np.float32)
    W_neigh = np.ascontiguousarray(W_neigh, dtype=np.float32)

    k_max, srcidx, dlarr, efarr, hsT = preprocess(
        h_neigh, h_self, edge_feats, src, dst
    )

    if k_max not in _PROGRAM_CACHE:
        _PROGRAM_CACHE[k_max] = build_program(k_max)
    nc = _PROGRAM_CACHE[k_max]

    wsT = np.ascontiguousarray(W_self.T)  # [128, 256]
    wnT1 = np.ascontiguousarray(W_neigh[:, :D_NEIGH].T)  # [128, 256]
    wnT2 = np.ascontiguousarray(W_neigh[:, D_NEIGH:].T)  # [32, 256]
    iota = np.tile(np.arange(P, dtype=np.float32), (P, 1))
    ident = np.eye(P, dtype=np.float32)

    in_maps = []
    for c in range(N_CORES):
        in_maps.append(
            {
                "h_neigh": h_neigh,
                "srcidx": srcidx[c],
                "dloc": dlarr[c],
                "ef": efarr[c],
                "h_selfT": hsT[c],
                "wsT": wsT,
                "wnT1": wnT1,
                "wnT2": wnT2,
                "iota": iota,
                "ident": ident,
            }
        )

    res = run_bass_kernel_spmd(nc, in_maps, list(range(N_CORES)))
    LAST_EXEC_NS = res.exec_time_ns

    out = np.empty((N_DST, D_OUT), dtype=np.float32)
    for c in range(N_CORES):
        out[c * DST_PER_CORE : (c + 1) * DST_PER_CORE] = res.results[c]["out"][
            :DST_PER_CORE
        ]
    return out

